# revision 12
# baseline (speedup 1.0000x reference)
"""MoE layer (8 experts, top-2) on 8 Trainium2 NeuronCores, expert-parallel.

Strategy:
  - Host computes the (tiny) gating linear + softmax + top-2 routing,
    mirroring the reference ops so expert selection matches exactly.
  - Tokens are dispatched to their experts on the host (the all-to-all),
    one expert per NeuronCore. Each core runs the 1024->4096->1024 gelu
    FFN for its expert over its routed tokens (padded to a common
    capacity), with all matmuls in float32r at full PE rate.
  - Host applies combine weights and scatter-adds back to token order.

Device layout: activations are kept transposed ([feature, token]) so both
matmuls consume the natural [K, M] weight layout and the phase-1 output
feeds phase-2 without any on-chip transpose. The 4096-wide hidden dim is
processed in quarters: phase 1 produces a quarter of the hidden
activations for ALL tokens (SBUF-resident), phase 2 immediately folds it
into an SBUF-resident partial sum of y. Expert weights therefore stream
from HBM exactly once, which keeps the kernel tensor-engine-bound
(streaming them per token-group was HBM-bound under 8-core contention).
"""

import numpy as np

N_EMBED = 1024
FFN_HIDDEN = 4096
NUM_EXPERTS = 8
TOP_K = 2
P = 128
KT1 = N_EMBED // P       # 8  k-tiles, phase 1
MT1 = FFN_HIDDEN // P    # 32 m-tiles, phase 1
KT2 = FFN_HIDDEN // P    # 32 k-tiles, phase 2
MT2 = N_EMBED // P       # 8  m-tiles, phase 2
QF = 4                   # FFN hidden dim is processed in QF f-quarters so the
                         # resident hT slab stays small and weights stream once

LAST_RESULT = None       # BassKernelResults of the most recent run (debug/profiling)


def _ensure_axon_hooks():
    """Make `antenv.axon_hooks` importable so BASS_TRACE=1 degrades
    gracefully instead of crashing when the image lacks the module."""
    try:
        import antenv.axon_hooks  # noqa: F401
        return
    except ImportError:
        pass
    import sys
    import types

    m = types.ModuleType("antenv.axon_hooks")
    m._hook = None
    m.set_axon_ntff_profile_hook = lambda h: setattr(m, "_hook", h)
    m.get_axon_ntff_profile_hook = lambda: m._hook
    sys.modules["antenv.axon_hooks"] = m
    try:
        from trn_agent_boot.trn_boot import _ntff_profile_via_ctypes

        m.set_axon_ntff_profile_hook(_ntff_profile_via_ctypes("/opt/axon/libaxon_pjrt.so"))
    except Exception:
        pass


def _route(x2d, Wg, bg):
    """Top-2 gating. Mirrors the reference (jax softmax + lax.top_k) so the
    selected experts match it exactly; numpy fallback is numerically
    equivalent up to fp32 rounding."""
    try:
        import jax
        import jax.numpy as jnp

        gate = jax.nn.softmax(jnp.asarray(x2d) @ jnp.asarray(Wg) + jnp.asarray(bg), axis=-1)
        scores, idx = jax.lax.top_k(gate, TOP_K)
        scores = np.asarray(scores, dtype=np.float32)
        idx = np.asarray(idx)
    except Exception:
        logits = x2d @ Wg + bg
        m = logits.max(-1, keepdims=True)
        e = np.exp(logits - m)
        p = e / e.sum(-1, keepdims=True)
        order = np.argsort(-p, axis=-1, kind="stable")
        idx = order[:, :TOP_K]
        scores = np.take_along_axis(p, idx, axis=-1)
    scores = scores / (scores.sum(-1, keepdims=True) + 1e-8)
    return idx.astype(np.int64), scores.astype(np.float32)


def _chunks(width):
    """Split the token capacity (>=256) into matmul free-dim chunks, each
    in [256, 512] so every fp32r matmul runs at full PE rate. Smallest
    chunk first: it gates the very first matmul of the kernel."""
    out, rem = [], width
    while rem > 0:
        if rem <= 512:
            c = rem
        elif rem < 768:
            c = rem - 256
        else:
            c = 512
        out.append(c)
        rem -= c
    out.sort()
    res, off = [], 0
    for c in out:
        res.append((off, c))
        off += c
    return res




def _build_device_program(cap, use_bf16=True):
    import concourse.tile as tile
    from concourse import bacc, mybir
    from concourse.tile_rust import add_dep_helper

    f32 = mybir.dt.float32
    f32r = mybir.dt.float32r
    # bf16 everywhere: same 1 col/cycle PE rate as fp32r, but half the HBM
    # traffic (the fp32 kernel ran at ~90% HBM utilization under 8-core
    # contention) and fast-weight-load LDWEIGHTS (~53ns vs ~191ns for the
    # 4-byte path), which was partially exposed per matmul in fp32r.
    mmdt = mybir.dt.bfloat16 if use_bf16 else f32r
    p2dt = mmdt
    gelu = mybir.ActivationFunctionType.Gelu_apprx_tanh
    ident = mybir.ActivationFunctionType.Identity

    nc = bacc.Bacc("TRN2", target_bir_lowering=False, debug=False,
                   num_devices=NUM_EXPERTS)

    MQ1 = MT1 // QF  # phase-1 m-tiles (f-tiles) per quarter
    KQ2 = KT2 // QF  # phase-2 k-tiles (f-tiles) per quarter

    # x is laid out [P, KT1, cap] so the whole first chunk (all k-tiles)
    # arrives via a single dma_start — 8 separate issues (~650ns each on the
    # sync sequencer) were serializing the kernel prologue
    xg_d = nc.dram_tensor("xg", [P, KT1, cap], mmdt, kind="ExternalInput").ap()
    w1_d = nc.dram_tensor("w1t", [MT1, P, KT1 * P], mmdt, kind="ExternalInput").ap()
    # w2 is laid out quarter-sliced: tile (q*MT2 + m) holds the KQ2 k-slices
    # of f-quarter q for output tile m
    w2_d = nc.dram_tensor("w2t", [QF * MT2, P, KQ2 * P], p2dt,
                          kind="ExternalInput").ap()
    b1_d = nc.dram_tensor("b1m", [P, MT1], f32, kind="ExternalInput").ap()
    b2_d = nc.dram_tensor("b2m", [P, MT2], f32, kind="ExternalInput").ap()
    y_d = nc.dram_tensor("yT", [MT2, P, cap], mmdt, kind="ExternalOutput").ap()

    chunks = _chunks(cap)

    with tile.TileContext(nc) as tc:
        with (
            tc.tile_pool(name="const", bufs=1) as const,
            tc.tile_pool(name="xp", bufs=1) as xp,
            tc.tile_pool(name="hp", bufs=1) as hp,
            tc.tile_pool(name="yp", bufs=1) as yp,
            tc.tile_pool(name="w1p", bufs=4) as w1p,
            tc.tile_pool(name="w2p", bufs=4) as w2p,
            tc.tile_pool(name="psp", bufs=8, space="PSUM") as psp,
            tc.tile_pool(name="op", bufs=6) as op,
        ):
            # single x tile [P, KT1, cap]; the first chunk (all k-tiles) is one
            # DMA so the first k-accumulation starts as soon as possible, the
            # remainder follows as a second DMA
            first_cw = chunks[0][1]
            xg_sb = xp.tile([P, KT1, cap], mmdt, name="xg")
            nc.sync.dma_start(xg_sb[:, :, :first_cw], xg_d[:, :, :first_cw])
            b1_sb = const.tile([P, MT1], f32)
            nc.sync.dma_start(b1_sb[:], b1_d[:, :])
            b2_sb = const.tile([P, MT2], f32)
            nc.sync.dma_start(b2_sb[:], b2_d[:, :])
            nc.sync.dma_start(xg_sb[:, :, first_cw:], xg_d[:, :, first_cw:])

            hT_sb = hp.tile([P, MQ1 * cap], p2dt)
            y_sb = yp.tile([P, MT2 * cap], f32)

            anchor_act = None
            for q in range(QF):
                # phase 1 (quarter q): hT = gelu(W1[:, fq].T @ xT + b1[fq])
                for mq in range(MQ1):
                    m = q * MQ1 + mq
                    w1m = w1p.tile([P, KT1 * P], mmdt, tag="w1")
                    nc.gpsimd.dma_start(w1m[:], w1_d[m, :, :])
                    for ci, (cs, cw) in enumerate(chunks):
                        ps = psp.tile([P, cw], f32, tag="ps", name=f"ps{ci}")
                        for kt in range(KT1):
                            nc.tensor.matmul(
                                ps[:],
                                w1m[:, kt * P:(kt + 1) * P],
                                xg_sb[:, kt, cs:cs + cw],
                                start=(kt == 0),
                                stop=(kt == KT1 - 1),
                            )
                        act = nc.scalar.activation(
                            hT_sb[:, mq * cap + cs:mq * cap + cs + cw],
                            ps[:],
                            gelu,
                            bias=b1_sb[:, m:m + 1],
                        )
                        if q == 0 and mq == 6 and ci == 0:
                            anchor_act = act.ins
                # phase 2 (quarter q): y (+)= W2[fq].T @ hT  [+ b2 on q=0]
                for m in range(MT2):
                    w2m = w2p.tile([P, KQ2 * P], p2dt, tag="w2")
                    w2dma = nc.gpsimd.dma_start(w2m[:], w2_d[q * MT2 + m, :, :])
                    if q == 0 and m < 2 and anchor_act is not None:
                        # keep w2 prefetches out of the prologue DMA queues;
                        # they are only needed once phase 1 is well underway
                        add_dep_helper(w2dma.ins, anchor_act, sync=False,
                                       reason="delay w2 prefetch past early phase-1")
                    # on the very last output tile, finish with the smallest
                    # chunk: its evacuate+store is the kernel's tail
                    mchunks = chunks
                    if q == QF - 1 and m == MT2 - 1:
                        mchunks = sorted(chunks, key=lambda c: -c[1])
                    for ci, (cs, cw) in enumerate(mchunks):
                        ps = psp.tile([P, cw], f32, tag="ps", name=f"ps{ci}")
                        for kq in range(KQ2):
                            nc.tensor.matmul(
                                ps[:],
                                w2m[:, kq * P:(kq + 1) * P],
                                hT_sb[:, kq * cap + cs:kq * cap + cs + cw],
                                start=(kq == 0),
                                stop=(kq == KQ2 - 1),
                            )
                        ysl = y_sb[:, m * cap + cs:m * cap + cs + cw]
                        if q == 0:
                            nc.scalar.activation(ysl, ps[:], ident,
                                                 bias=b2_sb[:, m:m + 1])
                        elif q < QF - 1:
                            nc.vector.tensor_add(ysl, ps[:], ysl)
                        else:
                            ot = op.tile([P, cw], mmdt, tag="o", name=f"o{ci}")
                            nc.vector.tensor_add(ot[:], ps[:], ysl)
                            nc.sync.dma_start(y_d[m, :, cs:cs + cw], ot[:])

    nc.compile()
    return nc


def kernel(x, Wg, bg, W1, b1, W2, b2):
    global LAST_RESULT
    _ensure_axon_hooks()
    from concourse.bass_utils import run_bass_kernel_spmd

    x = np.ascontiguousarray(np.asarray(x, dtype=np.float32))
    Wg = np.asarray(Wg, dtype=np.float32)
    bg = np.asarray(bg, dtype=np.float32)
    W1 = np.asarray(W1, dtype=np.float32)
    b1 = np.asarray(b1, dtype=np.float32)
    W2 = np.asarray(W2, dtype=np.float32)
    b2 = np.asarray(b2, dtype=np.float32)

    B, S, D = x.shape
    T = B * S
    xf = x.reshape(T, D)

    top_idx, top_w = _route(xf, Wg, bg)

    tok_idx = []
    tok_w = []
    for e in range(NUM_EXPERTS):
        sel = top_idx == e                       # [T, K]
        rows = np.nonzero(sel.any(axis=1))[0]
        tok_idx.append(rows)
        tok_w.append((top_w * sel).sum(axis=1)[rows].astype(np.float32))

    maxc = max(len(r) for r in tok_idx)
    cap = max(256, -(-maxc // 16) * 16)  # 64B-aligned rows, minimal padding

    import os as _os
    use_bf16 = not bool(_os.environ.get("MOE_FP32"))
    nc = _build_device_program(cap, use_bf16)

    import ml_dtypes
    mmdt_np = ml_dtypes.bfloat16 if use_bf16 else np.float32

    in_maps = []
    for e in range(NUM_EXPERTS):
        idx_pad = np.zeros(cap, dtype=np.int64)
        idx_pad[:len(tok_idx[e])] = tok_idx[e]
        # [P, KT1, cap]: partition-major so the device sees one strided DMA
        xg = np.ascontiguousarray(
            xf[idx_pad].T.reshape(KT1, P, cap).transpose(1, 0, 2))
        w1t = np.ascontiguousarray(
            W1[e].reshape(KT1, P, MT1, P).transpose(2, 1, 0, 3)
        ).reshape(MT1, P, KT1 * P)
        w2t = np.ascontiguousarray(
            W2[e].reshape(QF, KT2 // QF, P, MT2, P).transpose(0, 3, 2, 1, 4)
        ).reshape(QF * MT2, P, (KT2 // QF) * P)
        in_maps.append({
            "xg": xg.astype(mmdt_np),
            "w1t": w1t.astype(mmdt_np),
            "w2t": w2t.astype(mmdt_np),
            "b1m": np.ascontiguousarray(b1[e].reshape(MT1, P).T),
            "b2m": np.ascontiguousarray(b2[e].reshape(MT2, P).T),
        })

    import os
    trace_cores = None
    if os.environ.get("MOE_TRACE_ALL"):
        trace_cores = list(range(NUM_EXPERTS))
    res = run_bass_kernel_spmd(nc, in_maps, core_ids=list(range(NUM_EXPERTS)),
                               trace_cores=trace_cores)
    LAST_RESULT = res

    out = np.zeros((T, D), dtype=np.float32)
    for e in range(NUM_EXPERTS):
        n_e = len(tok_idx[e])
        if n_e == 0:
            continue
        yT = np.asarray(res.results[e]["yT"], dtype=np.float32).reshape(D, cap)
        out[tok_idx[e]] += tok_w[e][:, None] * yT[:, :n_e].T
    return out.reshape(B, S, D)



# revision 13
# speedup vs baseline: 1.0613x; 1.0613x over previous
"""MoE layer (8 experts, top-2) on 8 Trainium2 NeuronCores, pair-split
expert-parallel for load balance.

Strategy:
  - Host computes gating + top-2 routing (mirrors the reference ops).
  - Experts are sorted by routed-token count and split into the 4 heaviest
    ("A") and 4 lightest ("B"); pair i = (A_i, B_i) is assigned to the core
    pair (2i, 2i+1). Core 2i holds the FIRST half of the FFN hidden dim
    (f 0:2048) of BOTH its experts, core 2i+1 the second half. Both cores
    process ALL of the pair's tokens, each producing a partial y (its
    hidden-half contribution); the host sums the two partials and adds b2.
  - Per-core work is (capA + capB) * 256 PE cycles instead of
    2 * pad(max_load) * 256 — the padding waste of per-expert capacity is
    replaced by pad(1st) + pad(5th) of the sorted loads, which is nearly
    perfectly balanced. Weight DMA per core is unchanged (half the hidden
    dim of two experts == one full expert); only x and y DMA double.
  - Everything that touches the PE is bfloat16 (same 1 col/cycle rate as
    fp32r, half the HBM traffic, fast-weight-load LDWEIGHTS); PSUM
    accumulation is fp32; rel err ~4e-3 vs the 2e-2 gate.

Device layout: activations are kept transposed ([feature, token]) so both
matmuls consume the natural [K, M] weight layout and the phase-1 output
feeds phase-2 without any on-chip transpose. Per segment the full hidden
half's gelu activations stay SBUF-resident, so phase 2 accumulates all 16
k-tiles in one PSUM group and writes y directly to HBM (no on-chip y
accumulator). Expert weights stream from HBM exactly once.
"""

import numpy as np

N_EMBED = 1024
FFN_HIDDEN = 4096
NUM_EXPERTS = 8
TOP_K = 2
P = 128
KT1 = N_EMBED // P        # 8  k-tiles, phase 1
FH = FFN_HIDDEN // 2      # 2048 hidden per core (half)
MT1 = FH // P             # 16 m-tiles, phase 1 (per half)
KT2 = FH // P             # 16 k-tiles, phase 2 (per half)
MT2 = N_EMBED // P        # 8  m-tiles, phase 2

LAST_RESULT = None        # BassKernelResults of the most recent run


def _ensure_axon_hooks():
    """Make `antenv.axon_hooks` importable so BASS_TRACE=1 degrades
    gracefully instead of crashing when the image lacks the module."""
    try:
        import antenv.axon_hooks  # noqa: F401
        return
    except ImportError:
        pass
    import sys
    import types

    m = types.ModuleType("antenv.axon_hooks")
    m._hook = None
    m.set_axon_ntff_profile_hook = lambda h: setattr(m, "_hook", h)
    m.get_axon_ntff_profile_hook = lambda: m._hook
    sys.modules["antenv.axon_hooks"] = m
    try:
        from trn_agent_boot.trn_boot import _ntff_profile_via_ctypes

        m.set_axon_ntff_profile_hook(_ntff_profile_via_ctypes("/opt/axon/libaxon_pjrt.so"))
    except Exception:
        pass


def _route(x2d, Wg, bg):
    """Top-2 gating. Mirrors the reference (jax softmax + lax.top_k) so the
    selected experts match it exactly; numpy fallback is numerically
    equivalent up to fp32 rounding."""
    try:
        import jax
        import jax.numpy as jnp

        gate = jax.nn.softmax(jnp.asarray(x2d) @ jnp.asarray(Wg) + jnp.asarray(bg), axis=-1)
        scores, idx = jax.lax.top_k(gate, TOP_K)
        scores = np.asarray(scores, dtype=np.float32)
        idx = np.asarray(idx)
    except Exception:
        logits = x2d @ Wg + bg
        m = logits.max(-1, keepdims=True)
        e = np.exp(logits - m)
        p = e / e.sum(-1, keepdims=True)
        order = np.argsort(-p, axis=-1, kind="stable")
        idx = order[:, :TOP_K]
        scores = np.take_along_axis(p, idx, axis=-1)
    scores = scores / (scores.sum(-1, keepdims=True) + 1e-8)
    return idx.astype(np.int64), scores.astype(np.float32)


def _chunks(width):
    """Split a token capacity (>=256) into matmul free-dim chunks: a 128-col
    opener (so the very first accumulation group only waits on a 0.25MB x
    transfer) followed by chunks in [256, 512] (PSUM bank limit is 512
    fp32), smallest first."""
    out, rem = [128], width - 128
    while rem > 0:
        if rem <= 512:
            c = rem
        elif rem < 768:
            c = rem - 256
        else:
            c = 512
        out.append(c)
        rem -= c
    out = [out[0]] + sorted(out[1:])
    res, off = [], 0
    for c in out:
        res.append((off, c))
        off += c
    return res


def _build_device_program(capA, capB):
    import concourse.tile as tile
    from concourse import bacc, mybir
    from concourse.tile_rust import add_dep_helper

    f32 = mybir.dt.float32
    bf16 = mybir.dt.bfloat16
    gelu = mybir.ActivationFunctionType.Gelu_apprx_tanh
    ident = mybir.ActivationFunctionType.Identity

    nc = bacc.Bacc("TRN2", target_bir_lowering=False, debug=False,
                   num_devices=NUM_EXPERTS)

    segs = [("A", capA), ("B", capB)]
    dram = {}
    for s, cap in segs:
        dram[s] = {
            # [P, KT1, cap] so the whole first chunk is a single dma_start
            "xg": nc.dram_tensor(f"xg{s}", [P, KT1, cap], bf16,
                                 kind="ExternalInput").ap(),
            "w1": nc.dram_tensor(f"w1t{s}", [MT1, P, KT1 * P], bf16,
                                 kind="ExternalInput").ap(),
            "w2": nc.dram_tensor(f"w2t{s}", [MT2, P, KT2 * P], bf16,
                                 kind="ExternalInput").ap(),
            "b1": nc.dram_tensor(f"b1m{s}", [P, MT1], f32,
                                 kind="ExternalInput").ap(),
            "y": nc.dram_tensor(f"yT{s}", [MT2, P, cap], bf16,
                                kind="ExternalOutput").ap(),
        }

    with tile.TileContext(nc) as tc:
        with (
            tc.tile_pool(name="const", bufs=1) as const,
            tc.tile_pool(name="xp", bufs=1) as xp,
            tc.tile_pool(name="hp", bufs=2) as hp,
            tc.tile_pool(name="w1p", bufs=4) as w1p,
            tc.tile_pool(name="w2p", bufs=3) as w2p,
            tc.tile_pool(name="psp", bufs=8, space="PSUM") as psp,
            tc.tile_pool(name="op", bufs=6) as op,
        ):
            chunksA = _chunks(capA)
            chunksB = _chunks(capB)

            # prologue: segment A's first chunk gates the first matmul; issue
            # it first (per k-tile, so the transfer spreads across DMA
            # queues), then biases, then the rest of x
            xgA = xp.tile([P, KT1, capA], bf16, name="xgA")
            fcA = chunksA[0][1]
            for kt in range(KT1):
                nc.sync.dma_start(xgA[:, kt, :fcA], dram["A"]["xg"][:, kt, :fcA])
            b1A = const.tile([P, MT1], f32, name="b1A")
            nc.sync.dma_start(b1A[:], dram["A"]["b1"][:, :])
            b1B = const.tile([P, MT1], f32, name="b1B")
            nc.sync.dma_start(b1B[:], dram["B"]["b1"][:, :])
            for kt in range(KT1):
                nc.sync.dma_start(xgA[:, kt, fcA:], dram["A"]["xg"][:, kt, fcA:])
            xgB = xp.tile([P, KT1, capB], bf16, name="xgB")
            for kt in range(KT1):
                nc.sync.dma_start(xgB[:, kt, :], dram["B"]["xg"][:, kt, :])

            seg_in = {"A": (xgA, b1A, chunksA, capA),
                      "B": (xgB, b1B, chunksB, capB)}

            # anchor instructions used to keep the next weight stream's
            # first DMAs out of the current phase's DMA queues
            anchors = {}

            for si, (s, cap) in enumerate(segs):
                xg_sb, b1_sb, chunks, _ = seg_in[s]
                d = dram[s]

                # ---- phase 1: hT = gelu(W1h.T @ xT + b1h), SBUF-resident
                hT = hp.tile([P, MT1 * cap], bf16, tag="hT", name=f"hT{s}")
                for m in range(MT1):
                    w1m = w1p.tile([P, KT1 * P], bf16, tag="w1")
                    w1dma = nc.gpsimd.dma_start(w1m[:], d["w1"][m, :, :])
                    if s == "B" and m < 2 and "p2A" in anchors:
                        add_dep_helper(w1dma.ins, anchors["p2A"], sync=False,
                                       reason="delay w1B prefetch into phase-2 A")
                    for ci, (cs, cw) in enumerate(chunks):
                        ps = psp.tile([P, cw], f32, tag="ps", name=f"ps{ci}")
                        for kt in range(KT1):
                            nc.tensor.matmul(
                                ps[:],
                                w1m[:, kt * P:(kt + 1) * P],
                                xg_sb[:, kt, cs:cs + cw],
                                start=(kt == 0),
                                stop=(kt == KT1 - 1),
                            )
                        act = nc.scalar.activation(
                            hT[:, m * cap + cs:m * cap + cs + cw],
                            ps[:],
                            gelu,
                            bias=b1_sb[:, m:m + 1],
                        )
                        if m == 4 and ci == 0:
                            anchors[f"p1{s}"] = act.ins

                # ---- phase 2: y = W2h.T @ hT (no bias; host adds b2)
                for m in range(MT2):
                    w2m = w2p.tile([P, KT2 * P], bf16, tag="w2")
                    w2dma = nc.gpsimd.dma_start(w2m[:], d["w2"][m, :, :])
                    if m < 2 and f"p1{s}" in anchors:
                        add_dep_helper(w2dma.ins, anchors[f"p1{s}"], sync=False,
                                       reason="delay w2 prefetch past early phase-1")
                    # on the final output tile of the last segment, finish
                    # with the smallest chunk: its evacuate+store is the tail
                    mchunks = chunks
                    if si == len(segs) - 1 and m == MT2 - 1:
                        mchunks = sorted(chunks, key=lambda c: -c[1])
                    for ci, (cs, cw) in enumerate(mchunks):
                        ps = psp.tile([P, cw], f32, tag="ps", name=f"ps{ci}")
                        for kq in range(KT2):
                            nc.tensor.matmul(
                                ps[:],
                                w2m[:, kq * P:(kq + 1) * P],
                                hT[:, kq * cap + cs:cs + kq * cap + cw],
                                start=(kq == 0),
                                stop=(kq == KT2 - 1),
                            )
                        ot = op.tile([P, cw], bf16, tag="o", name=f"o{ci}")
                        act = nc.scalar.activation(ot[:], ps[:], ident)
                        if m == 2 and ci == 0:
                            anchors[f"p2{s}"] = act.ins
                        nc.sync.dma_start(d["y"][m, :, cs:cs + cw], ot[:])

    nc.compile()
    return nc


def kernel(x, Wg, bg, W1, b1, W2, b2):
    global LAST_RESULT
    _ensure_axon_hooks()
    from concourse.bass_utils import run_bass_kernel_spmd
    import ml_dtypes

    bf = ml_dtypes.bfloat16

    x = np.ascontiguousarray(np.asarray(x, dtype=np.float32))
    Wg = np.asarray(Wg, dtype=np.float32)
    bg = np.asarray(bg, dtype=np.float32)
    W1 = np.asarray(W1, dtype=np.float32)
    b1 = np.asarray(b1, dtype=np.float32)
    W2 = np.asarray(W2, dtype=np.float32)
    b2 = np.asarray(b2, dtype=np.float32)

    B, S, D = x.shape
    T = B * S
    xf = x.reshape(T, D)

    top_idx, top_w = _route(xf, Wg, bg)

    tok_idx = []
    tok_w = []
    for e in range(NUM_EXPERTS):
        sel = top_idx == e                       # [T, K]
        rows = np.nonzero(sel.any(axis=1))[0]
        tok_idx.append(rows)
        tok_w.append((top_w * sel).sum(axis=1)[rows].astype(np.float32))

    loads = np.array([len(r) for r in tok_idx])
    order = np.argsort(-loads, kind="stable")
    A_experts = [int(order[i]) for i in range(4)]         # 4 heaviest
    B_experts = [int(order[7 - i]) for i in range(4)]     # paired lightest
    capA = max(256, -(-int(loads[order[0]]) // 16) * 16)
    capB = max(256, -(-int(loads[order[4]]) // 16) * 16)

    nc = _build_device_program(capA, capB)

    def seg_arrays(e, cap, half):
        idx_pad = np.zeros(cap, dtype=np.int64)
        idx_pad[:len(tok_idx[e])] = tok_idx[e]
        # [P, KT1, cap]: partition-major so the device sees one strided DMA
        xg = np.ascontiguousarray(
            xf[idx_pad].T.reshape(KT1, P, cap).transpose(1, 0, 2)).astype(bf)
        w1h = W1[e][:, half * FH:(half + 1) * FH]          # [D, FH]
        w1t = np.ascontiguousarray(
            w1h.reshape(KT1, P, MT1, P).transpose(2, 1, 0, 3)
        ).reshape(MT1, P, KT1 * P).astype(bf)
        w2h = W2[e][half * FH:(half + 1) * FH, :]          # [FH, D]
        w2t = np.ascontiguousarray(
            w2h.reshape(KT2, P, MT2, P).transpose(2, 1, 0, 3)
        ).reshape(MT2, P, KT2 * P).astype(bf)
        b1h = b1[e][half * FH:(half + 1) * FH]
        b1m = np.ascontiguousarray(b1h.reshape(MT1, P).T)
        return xg, w1t, w2t, b1m

    in_maps = []
    for c in range(NUM_EXPERTS):
        pair, half = divmod(c, 2)
        eA, eB = A_experts[pair], B_experts[pair]
        xgA, w1A, w2A, b1A = seg_arrays(eA, capA, half)
        xgB, w1B, w2B, b1B = seg_arrays(eB, capB, half)
        in_maps.append({
            "xgA": xgA, "w1tA": w1A, "w2tA": w2A, "b1mA": b1A,
            "xgB": xgB, "w1tB": w1B, "w2tB": w2B, "b1mB": b1B,
        })

    import os
    trace_cores = None
    if os.environ.get("MOE_TRACE_ALL"):
        trace_cores = list(range(NUM_EXPERTS))
    res = run_bass_kernel_spmd(nc, in_maps, core_ids=list(range(NUM_EXPERTS)),
                               trace_cores=trace_cores)
    LAST_RESULT = res

    out = np.zeros((T, D), dtype=np.float32)
    for pair in range(4):
        c0, c1 = 2 * pair, 2 * pair + 1
        for key, e, cap in (("yTA", A_experts[pair], capA),
                            ("yTB", B_experts[pair], capB)):
            n_e = len(tok_idx[e])
            if n_e == 0:
                continue
            y0 = np.asarray(res.results[c0][key], dtype=np.float32)
            y1 = np.asarray(res.results[c1][key], dtype=np.float32)
            yT = (y0 + y1).reshape(D, cap)[:, :n_e]        # [D, n_e]
            y = yT.T + b2[e][None, :]
            out[tok_idx[e]] += tok_w[e][:, None] * y
    return out.reshape(B, S, D)


# revision 15
# speedup vs baseline: 1.0642x; 1.0027x over previous
"""MoE layer (8 experts, top-2) on 8 Trainium2 NeuronCores, pair-split
expert-parallel for load balance.

Strategy:
  - Host computes gating + top-2 routing (mirrors the reference ops).
  - Experts are sorted by routed-token count and split into the 4 heaviest
    ("A") and 4 lightest ("B"); pair i = (A_i, B_i) is assigned to the core
    pair (2i, 2i+1). Core 2i holds the FIRST half of the FFN hidden dim
    (f 0:2048) of BOTH its experts, core 2i+1 the second half. Both cores
    process ALL of the pair's tokens, each producing a partial y (its
    hidden-half contribution); the host sums the two partials and adds b2.
  - Per-core work is (capA + capB) * 256 PE cycles instead of
    2 * pad(max_load) * 256 — the padding waste of per-expert capacity is
    replaced by pad(1st) + pad(5th) of the sorted loads, which is nearly
    perfectly balanced. Weight DMA per core is unchanged (half the hidden
    dim of two experts == one full expert); only x and y DMA double.
  - Everything that touches the PE is bfloat16 (same 1 col/cycle rate as
    fp32r, half the HBM traffic, fast-weight-load LDWEIGHTS); PSUM
    accumulation is fp32; rel err ~4e-3 vs the 2e-2 gate.

Device layout: activations are kept transposed ([feature, token]) so both
matmuls consume the natural [K, M] weight layout and the phase-1 output
feeds phase-2 without any on-chip transpose. Per segment the full hidden
half's gelu activations stay SBUF-resident, so phase 2 accumulates all 16
k-tiles in one PSUM group and writes y directly to HBM (no on-chip y
accumulator). Expert weights stream from HBM exactly once.
"""

import numpy as np

N_EMBED = 1024
FFN_HIDDEN = 4096
NUM_EXPERTS = 8
TOP_K = 2
P = 128
KT1 = N_EMBED // P        # 8  k-tiles, phase 1
FH = FFN_HIDDEN // 2      # 2048 hidden per core (half)
MT1 = FH // P             # 16 m-tiles, phase 1 (per half)
KT2 = FH // P             # 16 k-tiles, phase 2 (per half)
MT2 = N_EMBED // P        # 8  m-tiles, phase 2

LAST_RESULT = None        # BassKernelResults of the most recent run


def _ensure_axon_hooks():
    """Make `antenv.axon_hooks` importable so BASS_TRACE=1 degrades
    gracefully instead of crashing when the image lacks the module."""
    try:
        import antenv.axon_hooks  # noqa: F401
        return
    except ImportError:
        pass
    import sys
    import types

    m = types.ModuleType("antenv.axon_hooks")
    m._hook = None
    m.set_axon_ntff_profile_hook = lambda h: setattr(m, "_hook", h)
    m.get_axon_ntff_profile_hook = lambda: m._hook
    sys.modules["antenv.axon_hooks"] = m
    try:
        from trn_agent_boot.trn_boot import _ntff_profile_via_ctypes

        m.set_axon_ntff_profile_hook(_ntff_profile_via_ctypes("/opt/axon/libaxon_pjrt.so"))
    except Exception:
        pass


def _route(x2d, Wg, bg):
    """Top-2 gating. Mirrors the reference (jax softmax + lax.top_k) so the
    selected experts match it exactly; numpy fallback is numerically
    equivalent up to fp32 rounding."""
    try:
        import jax
        import jax.numpy as jnp

        gate = jax.nn.softmax(jnp.asarray(x2d) @ jnp.asarray(Wg) + jnp.asarray(bg), axis=-1)
        scores, idx = jax.lax.top_k(gate, TOP_K)
        scores = np.asarray(scores, dtype=np.float32)
        idx = np.asarray(idx)
    except Exception:
        logits = x2d @ Wg + bg
        m = logits.max(-1, keepdims=True)
        e = np.exp(logits - m)
        p = e / e.sum(-1, keepdims=True)
        order = np.argsort(-p, axis=-1, kind="stable")
        idx = order[:, :TOP_K]
        scores = np.take_along_axis(p, idx, axis=-1)
    scores = scores / (scores.sum(-1, keepdims=True) + 1e-8)
    return idx.astype(np.int64), scores.astype(np.float32)


def _chunks(width):
    """Split a token capacity (>=256) into matmul free-dim chunks: a 128-col
    opener (so the very first accumulation group only waits on a 0.25MB x
    transfer) followed by chunks in [256, 512] (PSUM bank limit is 512
    fp32), smallest first."""
    out, rem = [128], width - 128
    while rem > 0:
        if rem <= 512:
            c = rem
        elif rem < 768:
            c = rem - 256
        else:
            c = 512
        out.append(c)
        rem -= c
    out = [out[0]] + sorted(out[1:])
    res, off = [], 0
    for c in out:
        res.append((off, c))
        off += c
    return res


def _build_device_program(capA, capB):
    import concourse.tile as tile
    from concourse import bacc, mybir
    from concourse.tile_rust import add_dep_helper

    f32 = mybir.dt.float32
    bf16 = mybir.dt.bfloat16
    gelu = mybir.ActivationFunctionType.Gelu_apprx_tanh
    ident = mybir.ActivationFunctionType.Identity

    nc = bacc.Bacc("TRN2", target_bir_lowering=False, debug=False,
                   num_devices=NUM_EXPERTS)

    segs = [("A", capA), ("B", capB)]
    dram = {}
    for s, cap in segs:
        dram[s] = {
            # [P, KT1, cap] so the whole first chunk is a single dma_start
            "xg": nc.dram_tensor(f"xg{s}", [P, KT1, cap], bf16,
                                 kind="ExternalInput").ap(),
            "w1": nc.dram_tensor(f"w1t{s}", [MT1, P, KT1 * P], bf16,
                                 kind="ExternalInput").ap(),
            "w2": nc.dram_tensor(f"w2t{s}", [MT2, P, KT2 * P], bf16,
                                 kind="ExternalInput").ap(),
            "b1": nc.dram_tensor(f"b1m{s}", [P, MT1], f32,
                                 kind="ExternalInput").ap(),
            "y": nc.dram_tensor(f"yT{s}", [MT2, P, cap], bf16,
                                kind="ExternalOutput").ap(),
        }

    with tile.TileContext(nc) as tc:
        with (
            tc.tile_pool(name="const", bufs=1) as const,
            tc.tile_pool(name="xp", bufs=1) as xp,
            tc.tile_pool(name="hp", bufs=2) as hp,
            tc.tile_pool(name="w1p", bufs=4) as w1p,
            tc.tile_pool(name="w2p", bufs=3) as w2p,
            tc.tile_pool(name="psp", bufs=8, space="PSUM") as psp,
            tc.tile_pool(name="op", bufs=6) as op,
        ):
            chunksA = _chunks(capA)
            chunksB = _chunks(capB)

            # prologue: segment A's first chunk gates the first matmul; issue
            # it first (per k-tile, so the transfer spreads across DMA
            # queues), then biases, then the rest of x.  Segment B's x is NOT
            # loaded here — it is emitted on the (otherwise idle) scalar DMA
            # queue after phase 1 A, keeping 2MB out of the congested
            # first ~15us where it caused PE stalls.
            xgA = xp.tile([P, KT1, capA], bf16, name="xgA")
            fcA = chunksA[0][1]
            for kt in range(KT1):
                nc.sync.dma_start(xgA[:, kt, :fcA], dram["A"]["xg"][:, kt, :fcA])
            b1A = const.tile([P, MT1], f32, name="b1A")
            nc.sync.dma_start(b1A[:], dram["A"]["b1"][:, :])
            b1B = const.tile([P, MT1], f32, name="b1B")
            nc.sync.dma_start(b1B[:], dram["B"]["b1"][:, :])
            for kt in range(KT1):
                nc.sync.dma_start(xgA[:, kt, fcA:], dram["A"]["xg"][:, kt, fcA:])
            xgB = xp.tile([P, KT1, capB], bf16, name="xgB")

            # HAM warmup: ~3.4us of junk matmuls (into a discarded PSUM slot)
            # run while the prologue DMAs are in flight, so the PE clock-gate
            # has flipped to full rate (2.4GHz) by the time real data lands.
            wjunk = const.tile([P, P], bf16, name="wjunk")
            nc.vector.memset(wjunk[:], 0)
            rjunk = const.tile([P, 512], bf16, name="rjunk")
            nc.vector.memset(rjunk[:], 0)
            wps = psp.tile([P, 512], f32, tag="ps", name="warm")
            NWARM = 8
            for i in range(NWARM):
                nc.tensor.matmul(wps[:], wjunk[:], rjunk[:],
                                 start=(i == 0), stop=(i == NWARM - 1))

            seg_in = {"A": (xgA, b1A, chunksA, capA),
                      "B": (xgB, b1B, chunksB, capB)}

            # anchor instructions used to keep the next weight stream's
            # first DMAs out of the current phase's DMA queues
            anchors = {}

            for si, (s, cap) in enumerate(segs):
                xg_sb, b1_sb, chunks, _ = seg_in[s]
                d = dram[s]

                # ---- phase 1: hT = gelu(W1h.T @ xT + b1h), SBUF-resident
                hT = hp.tile([P, MT1 * cap], bf16, tag="hT", name=f"hT{s}")
                for m in range(MT1):
                    w1m = w1p.tile([P, KT1 * P], bf16, tag="w1")
                    w1dma = nc.gpsimd.dma_start(w1m[:], d["w1"][m, :, :])
                    if s == "B" and m < 2 and "p2A" in anchors:
                        add_dep_helper(w1dma.ins, anchors["p2A"], sync=False,
                                       reason="delay w1B prefetch into phase-2 A")
                    for ci, (cs, cw) in enumerate(chunks):
                        ps = psp.tile([P, cw], f32, tag="ps", name=f"ps{ci}")
                        for kt in range(KT1):
                            nc.tensor.matmul(
                                ps[:],
                                w1m[:, kt * P:(kt + 1) * P],
                                xg_sb[:, kt, cs:cs + cw],
                                start=(kt == 0),
                                stop=(kt == KT1 - 1),
                            )
                        act = nc.scalar.activation(
                            hT[:, m * cap + cs:m * cap + cs + cw],
                            ps[:],
                            gelu,
                            bias=b1_sb[:, m:m + 1],
                        )
                        if m == 4 and ci == 0:
                            anchors[f"p1{s}"] = act.ins

                if s == "A":
                    # segment B's x: on the scalar HWDGE queue (idle), held
                    # past mid-phase-1 so it stays clear of the prologue burst
                    for kt in range(KT1):
                        xbd = nc.scalar.dma_start(xgB[:, kt, :],
                                                  dram["B"]["xg"][:, kt, :])
                        add_dep_helper(xbd.ins, anchors["p1A"], sync=False,
                                       reason="delay xgB load past early phase-1")

                # ---- phase 2: y = W2h.T @ hT (no bias; host adds b2)
                for m in range(MT2):
                    w2m = w2p.tile([P, KT2 * P], bf16, tag="w2")
                    w2dma = nc.gpsimd.dma_start(w2m[:], d["w2"][m, :, :])
                    if m < 2 and f"p1{s}" in anchors:
                        add_dep_helper(w2dma.ins, anchors[f"p1{s}"], sync=False,
                                       reason="delay w2 prefetch past early phase-1")
                    # on the final output tile of the last segment, finish
                    # with the smallest chunk: its evacuate+store is the tail
                    mchunks = chunks
                    if si == len(segs) - 1 and m == MT2 - 1:
                        mchunks = sorted(chunks, key=lambda c: -c[1])
                    for ci, (cs, cw) in enumerate(mchunks):
                        ps = psp.tile([P, cw], f32, tag="ps", name=f"ps{ci}")
                        for kq in range(KT2):
                            nc.tensor.matmul(
                                ps[:],
                                w2m[:, kq * P:(kq + 1) * P],
                                hT[:, kq * cap + cs:cs + kq * cap + cw],
                                start=(kq == 0),
                                stop=(kq == KT2 - 1),
                            )
                        ot = op.tile([P, cw], bf16, tag="o", name=f"o{ci}")
                        act = nc.scalar.activation(ot[:], ps[:], ident)
                        if m == 2 and ci == 0:
                            anchors[f"p2{s}"] = act.ins
                        nc.sync.dma_start(d["y"][m, :, cs:cs + cw], ot[:])

    nc.compile()
    return nc


def kernel(x, Wg, bg, W1, b1, W2, b2):
    global LAST_RESULT
    _ensure_axon_hooks()
    from concourse.bass_utils import run_bass_kernel_spmd
    import ml_dtypes

    bf = ml_dtypes.bfloat16

    x = np.ascontiguousarray(np.asarray(x, dtype=np.float32))
    Wg = np.asarray(Wg, dtype=np.float32)
    bg = np.asarray(bg, dtype=np.float32)
    W1 = np.asarray(W1, dtype=np.float32)
    b1 = np.asarray(b1, dtype=np.float32)
    W2 = np.asarray(W2, dtype=np.float32)
    b2 = np.asarray(b2, dtype=np.float32)

    B, S, D = x.shape
    T = B * S
    xf = x.reshape(T, D)

    top_idx, top_w = _route(xf, Wg, bg)

    tok_idx = []
    tok_w = []
    for e in range(NUM_EXPERTS):
        sel = top_idx == e                       # [T, K]
        rows = np.nonzero(sel.any(axis=1))[0]
        tok_idx.append(rows)
        tok_w.append((top_w * sel).sum(axis=1)[rows].astype(np.float32))

    loads = np.array([len(r) for r in tok_idx])
    order = np.argsort(-loads, kind="stable")
    A_experts = [int(order[i]) for i in range(4)]         # 4 heaviest
    B_experts = [int(order[7 - i]) for i in range(4)]     # paired lightest
    capA = max(256, -(-int(loads[order[0]]) // 16) * 16)
    capB = max(256, -(-int(loads[order[4]]) // 16) * 16)

    nc = _build_device_program(capA, capB)

    def seg_arrays(e, cap, half):
        idx_pad = np.zeros(cap, dtype=np.int64)
        idx_pad[:len(tok_idx[e])] = tok_idx[e]
        # [P, KT1, cap]: partition-major so the device sees one strided DMA
        xg = np.ascontiguousarray(
            xf[idx_pad].T.reshape(KT1, P, cap).transpose(1, 0, 2)).astype(bf)
        w1h = W1[e][:, half * FH:(half + 1) * FH]          # [D, FH]
        w1t = np.ascontiguousarray(
            w1h.reshape(KT1, P, MT1, P).transpose(2, 1, 0, 3)
        ).reshape(MT1, P, KT1 * P).astype(bf)
        w2h = W2[e][half * FH:(half + 1) * FH, :]          # [FH, D]
        w2t = np.ascontiguousarray(
            w2h.reshape(KT2, P, MT2, P).transpose(2, 1, 0, 3)
        ).reshape(MT2, P, KT2 * P).astype(bf)
        b1h = b1[e][half * FH:(half + 1) * FH]
        b1m = np.ascontiguousarray(b1h.reshape(MT1, P).T)
        return xg, w1t, w2t, b1m

    in_maps = []
    for c in range(NUM_EXPERTS):
        pair, half = divmod(c, 2)
        eA, eB = A_experts[pair], B_experts[pair]
        xgA, w1A, w2A, b1A = seg_arrays(eA, capA, half)
        xgB, w1B, w2B, b1B = seg_arrays(eB, capB, half)
        in_maps.append({
            "xgA": xgA, "w1tA": w1A, "w2tA": w2A, "b1mA": b1A,
            "xgB": xgB, "w1tB": w1B, "w2tB": w2B, "b1mB": b1B,
        })

    import os
    trace_cores = None
    if os.environ.get("MOE_TRACE_ALL"):
        trace_cores = list(range(NUM_EXPERTS))
    res = run_bass_kernel_spmd(nc, in_maps, core_ids=list(range(NUM_EXPERTS)),
                               trace_cores=trace_cores)
    LAST_RESULT = res

    out = np.zeros((T, D), dtype=np.float32)
    for pair in range(4):
        c0, c1 = 2 * pair, 2 * pair + 1
        for key, e, cap in (("yTA", A_experts[pair], capA),
                            ("yTB", B_experts[pair], capB)):
            n_e = len(tok_idx[e])
            if n_e == 0:
                continue
            y0 = np.asarray(res.results[c0][key], dtype=np.float32)
            y1 = np.asarray(res.results[c1][key], dtype=np.float32)
            yT = (y0 + y1).reshape(D, cap)[:, :n_e]        # [D, n_e]
            y = yT.T + b2[e][None, :]
            out[tok_idx[e]] += tok_w[e][:, None] * y
    return out.reshape(B, S, D)
